# revision 2
# baseline (speedup 1.0000x reference)
"""Fused LoRA-MLP (SwiGLU) expert kernel for TRN2, 8-core expert-parallel.

Problem (per full batch): x:(8192,2048) shared-weight expert MLP
    gu  = x @ W_gu.T + 0.25 * (x @ A_gu.T) @ B_gu.T        (.,8192)
    h   = gu[:, 4096:] * silu(gu[:, :4096])                 (.,4096)
    out = h @ W_d.T  + 0.25 * (h @ A_d.T)  @ B_d.T          (.,2048)

LoRA is a rank-64 delta on the weights, so it is folded on the host:
    W_gu_eff = W_gu + 0.25 * B_gu @ A_gu
    W_d_eff  = W_d  + 0.25 * B_d  @ A_d
leaving a pure dense SwiGLU MLP on device.

Sharding: expert/data parallel — core c owns tokens [1024c, 1024(c+1)),
weights replicated per core. No collectives.

All tensors are pre-transposed/pre-tiled on the host so the device kernel
needs zero on-chip transposes; activations flow feature-major
(xT -> guT -> hT -> outT). Matmuls run as float32r (full-rate fp32).
"""

import os
from contextlib import ExitStack

import numpy as np

import concourse.bass as bass
import concourse.bacc as bacc
import concourse.tile as tile
import concourse.mybir as mybir
from concourse.bass_utils import run_bass_kernel_spmd

F32 = mybir.dt.float32
F32R = mybir.dt.float32r
AF = mybir.ActivationFunctionType

NCORES = 8
T = 1024          # tokens per core
H = 2048          # hidden
D = 4096          # expert dim
F = 2 * D         # gate+up features
R = 64            # lora rank
SCALING = 16 / 64

KT = H // 128     # 16 k-tiles (mm1 contraction)
FT = F // 128     # 64 f-tiles (mm1 outputs)
DT = D // 128     # 32 d-tiles (mm2 contraction)
JT = H // 128     # 16 j-tiles (mm2 outputs)
NB = 8            # mm2 d-blocks (4 d-tiles each)
TC = 512          # moving-dim chunk (fp32 moving max)
NCH = T // TC     # 2 chunks
SLAB = KT * 128   # wgu slab cols (16 k-tiles)

_CACHE = {}


def _r(ap):
    return ap.bitcast(mybir.dt.float32r)


def _build(reps=1):
    nc = bacc.Bacc("TRN2", target_bir_lowering=False, debug=False,
                   num_devices=NCORES)

    xT = nc.dram_tensor("xT", [128, KT * T], F32R, kind="ExternalInput")
    wgu = nc.dram_tensor("wgu", [FT, 128, SLAB], F32R, kind="ExternalInput")
    wd = nc.dram_tensor("wd", [NB, JT, 128, 4 * 128], F32R, kind="ExternalInput")
    outT = nc.dram_tensor("outT", [JT, 128, T], F32, kind="ExternalOutput")

    with tile.TileContext(nc) as tc, ExitStack() as ctx:
        xpool = ctx.enter_context(tc.tile_pool(name="xpool", bufs=1))
        wgu_pool = ctx.enter_context(tc.tile_pool(name="wgup", bufs=3))
        wd_pool = ctx.enter_context(tc.tile_pool(name="wdp", bufs=3))
        ht_pool = ctx.enter_context(tc.tile_pool(name="htp", bufs=5))
        oacc_pool = ctx.enter_context(tc.tile_pool(name="oaccp", bufs=JT))
        sil_pool = ctx.enter_context(tc.tile_pool(name="silp", bufs=2))
        ps_a = ctx.enter_context(tc.tile_pool(name="psa", bufs=4, space="PSUM"))
        ps_b = ctx.enter_context(tc.tile_pool(name="psb", bufs=4, space="PSUM"))

        for rep in range(reps):
            # first mm1 weight slabs, then x streamed per k-tile so the PE
            # can start as soon as slab0 + the first x chunks land
            slab0 = {}
            for m in (0, DT):
                s = wgu_pool.tile([128, SLAB], F32, tag="wgu")
                nc.sync.dma_start(out=_r(s[:]), in_=wgu[m])
                slab0[m] = s

            xbuf = xpool.tile([128, KT * T], F32)
            for k in range(KT):
                nc.sync.dma_start(out=_r(xbuf[:, k * T:(k + 1) * T]),
                                  in_=xT[:, k * T:(k + 1) * T])

            def xsl(k, c):
                return xbuf[:, k * T + c * TC: k * T + (c + 1) * TC]

            ht_tiles = [None] * DT
            oacc = [None] * JT

            def mm2_block(b):
                for j in range(JT):
                    wdt = wd_pool.tile([128, 4 * 128], F32, tag="wd")
                    nc.sync.dma_start(out=_r(wdt[:]), in_=wd[b, j])
                    for c in range(NCH):
                        ps = ps_b.tile([128, TC], F32, tag="psb")
                        for dt_ in range(4):
                            d = b * 4 + dt_
                            nc.tensor.matmul(
                                ps[:], _r(wdt[:, dt_ * 128:(dt_ + 1) * 128]),
                                _r(ht_tiles[d][:, c * TC:(c + 1) * TC]),
                                start=(dt_ == 0), stop=(dt_ == 3))
                        dst = oacc[j][:, c * TC:(c + 1) * TC]
                        if b == 0:
                            nc.vector.tensor_copy(dst, ps[:])
                        else:
                            nc.vector.tensor_add(dst, dst, ps[:])
                            if b == NB - 1:
                                nc.sync.dma_start(
                                    out=outT[j, :, c * TC:(c + 1) * TC],
                                    in_=oacc[j][:, c * TC:(c + 1) * TC])

            # ---- main mm1 loop over f-pairs (gate m=i, up m=i+32)
            for i in range(DT):
                # mm2 lagged one block so PE never waits on fresh DVE output
                if i % 4 == 0 and i > 0:
                    mm2_block(i // 4 - 1)

                if i == 0:
                    slabs = slab0
                else:
                    slabs = {}
                    for m in (i, i + DT):
                        s = wgu_pool.tile([128, SLAB], F32, tag="wgu")
                        nc.sync.dma_start(out=_r(s[:]), in_=wgu[m])
                        slabs[m] = s

                ht_i = ht_pool.tile([128, T], F32, tag="ht")
                ht_tiles[i] = ht_i
                for c in range(NCH):
                    pg = ps_a.tile([128, TC], F32, tag="psa")
                    pu = ps_a.tile([128, TC], F32, tag="psa")
                    for ps, m in ((pg, i), (pu, i + DT)):
                        s = slabs[m]
                        for k in range(KT):
                            nc.tensor.matmul(
                                ps[:], _r(s[:, k * 128:(k + 1) * 128]),
                                _r(xsl(k, c)), start=(k == 0),
                                stop=(k == KT - 1))
                    sil = sil_pool.tile([128, TC], F32, tag="sil")
                    nc.scalar.activation(sil[:], pg[:], AF.Silu)
                    nc.vector.tensor_mul(_r(ht_i[:, c * TC:(c + 1) * TC]), pu[:], sil[:])

                if i == 0:
                    for j in range(JT):
                        oacc[j] = oacc_pool.tile([128, T], F32, tag="oacc",
                                                 name=f"oacc_{rep}_{j}")

            mm2_block(NB - 1)

    nc.compile()
    return nc


def _prep_shared(W_gu, A_gu, B_gu, W_d, A_d, B_d):
    # fold the rank-64 LoRA deltas into the dense weights
    wgu_eff = W_gu + SCALING * (B_gu @ A_gu)
    wd_eff = W_d + SCALING * (B_d @ A_d)
    # wgu slab [m, p, SLAB]: cols = W_gu_eff.T k-tiles for f-tile m
    wgu_t = np.ascontiguousarray(
        wgu_eff.reshape(FT, 128, KT, 128).transpose(0, 3, 2, 1)
    ).reshape(FT, 128, SLAB)
    wd_t = np.ascontiguousarray(
        wd_eff.reshape(JT, 128, NB, 4, 128).transpose(2, 0, 4, 3, 1)
    ).reshape(NB, JT, 128, 4 * 128)
    return dict(wgu=wgu_t, wd=wd_t)


def kernel(hidden_states, W_gu, A_gu, B_gu, W_d, A_d, B_d):
    hidden_states = np.asarray(hidden_states, dtype=np.float32)
    shared = _prep_shared(*(np.asarray(a, dtype=np.float32)
                            for a in (W_gu, A_gu, B_gu, W_d, A_d, B_d)))

    # per-core xT pre-tiled as [p, k, t] flattened to [128, KT*T]
    xt = np.ascontiguousarray(
        hidden_states.reshape(NCORES, T, KT, 128).transpose(0, 3, 2, 1)
    ).reshape(NCORES, 128, KT * T)

    if "nc" not in _CACHE:
        _CACHE["nc"] = _build()
    nc = _CACHE["nc"]

    in_maps = [dict(shared, xT=xt[c]) for c in range(NCORES)]
    trace = os.environ.get("KERNEL_TRACE", "0") == "1"
    res = run_bass_kernel_spmd(nc, in_maps, list(range(NCORES)), trace=trace)
    _CACHE["last_result"] = res

    out = np.empty((NCORES, T, H), np.float32)
    for c in range(NCORES):
        o = res.results[c]["outT"].reshape(JT, 128, T)
        out[c] = o.transpose(2, 0, 1).reshape(T, H)
    return out.reshape(NCORES * T, H)


# revision 5
# speedup vs baseline: 1.0644x; 1.0644x over previous
"""Fused LoRA-MLP (SwiGLU) expert kernel for TRN2, 8-core expert-parallel.

Problem (per full batch): x:(8192,2048) shared-weight expert MLP
    gu  = x @ W_gu.T + 0.25 * (x @ A_gu.T) @ B_gu.T        (.,8192)
    h   = gu[:, 4096:] * silu(gu[:, :4096])                 (.,4096)
    out = h @ W_d.T  + 0.25 * (h @ A_d.T)  @ B_d.T          (.,2048)

LoRA is a rank-64 delta on the weights, so it is folded on the host:
    W_gu_eff = W_gu + 0.25 * B_gu @ A_gu
    W_d_eff  = W_d  + 0.25 * B_d  @ A_d
leaving a pure dense SwiGLU MLP on device.

Sharding: expert/data parallel — core c owns tokens [1024c, 1024(c+1)),
weights replicated per core. No collectives.

Weights and activations stream as bf16 (matmul rate on TRN2 is
1 row/cycle for both bf16 and fp32r, but bf16 halves DMA bytes and
SBUF footprint); accumulation stays fp32 in PSUM, output is fp32.
All tensors are pre-transposed/pre-tiled on the host so the device
kernel needs zero on-chip transposes (xT -> guT -> hT -> outT).
"""

import os
from contextlib import ExitStack

import numpy as np
import ml_dtypes

import concourse.bass as bass
import concourse.bacc as bacc
import concourse.tile as tile
import concourse.mybir as mybir
from concourse.bass_utils import run_bass_kernel_spmd

F32 = mybir.dt.float32
BF16 = mybir.dt.bfloat16
AF = mybir.ActivationFunctionType
NPBF16 = ml_dtypes.bfloat16

NCORES = 8
T = 1024          # tokens per core
H = 2048          # hidden
D = 4096          # expert dim
F = 2 * D         # gate+up features
R = 64            # lora rank
SCALING = 16 / 64

KT = H // 128     # 16 k-tiles (mm1 contraction)
FT = F // 128     # 64 f-tiles (mm1 outputs)
DT = D // 128     # 32 d-tiles (mm2 contraction)
JT = H // 128     # 16 j-tiles (mm2 outputs)
NB = 8            # mm2 d-blocks (4 d-tiles each)
TC = 512          # moving-dim chunk (one PSUM bank)
NCH = T // TC     # 2 chunks
SLAB = KT * 128   # wgu slab cols (16 k-tiles)

_CACHE = {}


def _build(reps=1):
    nc = bacc.Bacc("TRN2", target_bir_lowering=False, debug=False,
                   num_devices=NCORES)

    xT = nc.dram_tensor("xT", [128, KT * T], BF16, kind="ExternalInput")
    wgu = nc.dram_tensor("wgu", [FT, 128, SLAB], BF16, kind="ExternalInput")
    wd = nc.dram_tensor("wd", [NB, JT, 128, 4 * 128], BF16, kind="ExternalInput")
    outT = nc.dram_tensor("outT", [JT, 128, T], F32, kind="ExternalOutput")

    with tile.TileContext(nc) as tc, ExitStack() as ctx:
        xpool = ctx.enter_context(tc.tile_pool(name="xpool", bufs=1))
        wgu_pool = ctx.enter_context(tc.tile_pool(name="wgup", bufs=3))
        wd_pool = ctx.enter_context(tc.tile_pool(name="wdp", bufs=33))
        ht_pool = ctx.enter_context(tc.tile_pool(name="htp", bufs=5))
        oacc_pool = ctx.enter_context(tc.tile_pool(name="oaccp", bufs=JT))
        sil_pool = ctx.enter_context(tc.tile_pool(name="silp", bufs=2))
        ps_a = ctx.enter_context(tc.tile_pool(name="psa", bufs=4, space="PSUM"))
        ps_b = ctx.enter_context(tc.tile_pool(name="psb", bufs=4, space="PSUM"))

        for rep in range(reps):
            # startup: first pair's slabs in halves interleaved with the
            # first x k-tiles so the PE starts ~2us in
            slab0 = {}
            for m in (0, DT):
                slab0[m] = wgu_pool.tile([128, SLAB], BF16, tag="wgu",
                                         name=f"slab0_{rep}_{m}")
            half = SLAB // 2
            for m in (0, DT):
                nc.sync.dma_start(out=slab0[m][:, :half], in_=wgu[m, :, :half])

            xbuf = xpool.tile([128, KT * T], BF16)

            def xsl(k, c):
                return xbuf[:, k * T + c * TC: k * T + (c + 1) * TC]

            nc.sync.dma_start(out=xbuf[:, :T], in_=xT[:, :T])
            for m in (0, DT):
                nc.sync.dma_start(out=slab0[m][:, half:], in_=wgu[m, :, half:])
            for k in range(1, KT):
                nc.sync.dma_start(out=xbuf[:, k * T:(k + 1) * T],
                                  in_=xT[:, k * T:(k + 1) * T])

            ht_tiles = [None] * DT
            oacc = [None] * JT
            wd_tiles = {}

            def emit_wd_dmas(b):
                tiles = []
                for j in range(JT):
                    wdt = wd_pool.tile([128, 4 * 128], BF16, tag="wd")
                    nc.sync.dma_start(out=wdt[:], in_=wd[b, j])
                    tiles.append(wdt)
                wd_tiles[b] = tiles

            def mm2_block(b):
                tiles = wd_tiles.pop(b)
                for j in range(JT):
                    wdt = tiles[j]
                    for c in range(NCH):
                        ps = ps_b.tile([128, TC], F32, tag="psb")
                        for dt_ in range(4):
                            d = b * 4 + dt_
                            nc.tensor.matmul(
                                ps[:], wdt[:, dt_ * 128:(dt_ + 1) * 128],
                                ht_tiles[d][:, c * TC:(c + 1) * TC],
                                start=(dt_ == 0), stop=(dt_ == 3))
                        dst = oacc[j][:, c * TC:(c + 1) * TC]
                        if b == 0:
                            nc.vector.tensor_copy(dst, ps[:])
                        else:
                            nc.vector.tensor_add(dst, dst, ps[:])
                            if b == NB - 1:
                                nc.sync.dma_start(
                                    out=outT[j, :, c * TC:(c + 1) * TC],
                                    in_=oacc[j][:, c * TC:(c + 1) * TC])

            # ---- main mm1 loop over f-pairs (gate m=i, up m=i+32)
            for i in range(DT):
                # mm2 lagged one block so PE never waits on fresh DVE output
                if i % 4 == 0 and i > 0:
                    mm2_block(i // 4 - 1)
                # wd tiles for block b prefetched two pairs before use
                if i % 4 == 2:
                    emit_wd_dmas(i // 4)

                if i == 0:
                    slabs = slab0
                else:
                    slabs = {}
                    for m in (i, i + DT):
                        s = wgu_pool.tile([128, SLAB], BF16, tag="wgu")
                        nc.sync.dma_start(out=s[:], in_=wgu[m])
                        slabs[m] = s

                ht_i = ht_pool.tile([128, T], BF16, tag="ht")
                ht_tiles[i] = ht_i
                if i == 0:
                    # k-outer: 4 concurrent PSUM groups consume x k-tiles
                    # as they stream in, so the PE is never x-starved
                    grp = {}
                    for c in range(NCH):
                        grp[(c, 0)] = ps_a.tile([128, TC], F32, tag="psa",
                                                name=f"grp_{rep}_{c}_0")
                        grp[(c, 1)] = ps_a.tile([128, TC], F32, tag="psa",
                                                name=f"grp_{rep}_{c}_1")
                    for k in range(KT):
                        for c in range(NCH):
                            for gi, m in ((0, i), (1, i + DT)):
                                nc.tensor.matmul(
                                    grp[(c, gi)][:],
                                    slabs[m][:, k * 128:(k + 1) * 128],
                                    xsl(k, c), start=(k == 0),
                                    stop=(k == KT - 1))
                    for c in range(NCH):
                        sil = sil_pool.tile([128, TC], F32, tag="sil")
                        nc.scalar.activation(sil[:], grp[(c, 0)][:], AF.Silu)
                        nc.vector.tensor_mul(ht_i[:, c * TC:(c + 1) * TC],
                                             grp[(c, 1)][:], sil[:])
                else:
                    for c in range(NCH):
                        pg = ps_a.tile([128, TC], F32, tag="psa")
                        pu = ps_a.tile([128, TC], F32, tag="psa")
                        for ps, m in ((pg, i), (pu, i + DT)):
                            s = slabs[m]
                            for k in range(KT):
                                nc.tensor.matmul(
                                    ps[:], s[:, k * 128:(k + 1) * 128],
                                    xsl(k, c), start=(k == 0),
                                    stop=(k == KT - 1))
                        sil = sil_pool.tile([128, TC], F32, tag="sil")
                        nc.scalar.activation(sil[:], pg[:], AF.Silu)
                        nc.vector.tensor_mul(ht_i[:, c * TC:(c + 1) * TC],
                                             pu[:], sil[:])

                if i == 0:
                    for j in range(JT):
                        oacc[j] = oacc_pool.tile([128, T], F32, tag="oacc",
                                                 name=f"oacc_{rep}_{j}")

            mm2_block(NB - 1)

    nc.compile()
    return nc


def _prep_shared(W_gu, A_gu, B_gu, W_d, A_d, B_d):
    # fold the rank-64 LoRA deltas into the dense weights
    wgu_eff = W_gu + SCALING * (B_gu @ A_gu)
    wd_eff = W_d + SCALING * (B_d @ A_d)
    # wgu slab [m, p, SLAB]: cols = W_gu_eff.T k-tiles for f-tile m
    wgu_t = np.ascontiguousarray(
        wgu_eff.reshape(FT, 128, KT, 128).transpose(0, 3, 2, 1)
    ).reshape(FT, 128, SLAB).astype(NPBF16)
    wd_t = np.ascontiguousarray(
        wd_eff.reshape(JT, 128, NB, 4, 128).transpose(2, 0, 4, 3, 1)
    ).reshape(NB, JT, 128, 4 * 128).astype(NPBF16)
    return dict(wgu=wgu_t, wd=wd_t)


def kernel(hidden_states, W_gu, A_gu, B_gu, W_d, A_d, B_d):
    hidden_states = np.asarray(hidden_states, dtype=np.float32)
    shared = _prep_shared(*(np.asarray(a, dtype=np.float32)
                            for a in (W_gu, A_gu, B_gu, W_d, A_d, B_d)))

    # per-core xT pre-tiled as [p, k, t] flattened to [128, KT*T]
    xt = np.ascontiguousarray(
        hidden_states.reshape(NCORES, T, KT, 128).transpose(0, 3, 2, 1)
    ).reshape(NCORES, 128, KT * T).astype(NPBF16)

    if "nc" not in _CACHE:
        _CACHE["nc"] = _build()
    nc = _CACHE["nc"]

    in_maps = [dict(shared, xT=xt[c]) for c in range(NCORES)]
    trace = os.environ.get("KERNEL_TRACE", "0") == "1"
    res = run_bass_kernel_spmd(nc, in_maps, list(range(NCORES)), trace=trace)
    _CACHE["last_result"] = res

    out = np.empty((NCORES, T, H), np.float32)
    for c in range(NCORES):
        o = res.results[c]["outT"].reshape(JT, 128, T)
        out[c] = o.transpose(2, 0, 1).reshape(T, H)
    return out.reshape(NCORES * T, H)
